# revision 26
# baseline (speedup 1.0000x reference)
"""Multi-head attention (batch=2, seq=2048, d_model=2048, 16 heads, causal)
on 8 Trainium2 NeuronCores.

Sharding (Megatron-style tensor parallel + data parallel):
  core c -> batch b = c // 4, feature block j = c % 4 (4 heads = 512 features).
  Each core computes Q/K/V projections for its 512 feature columns
  (w_q/w_k/w_v column-sliced), attention for its 4 heads, and a partial
  output projection (w_o row-sliced).  The 4 partial outputs per batch
  element are summed on the host (the Megatron row-parallel AllReduce).

Device math (per core), all matmuls in bf16 with fp32 PSUM accumulation
(bf16 measured faster than fp16 on silicon):
  xT  = x[b].T                          [2048 dm, 2048 s]   (host-prepped)
  Q^T = wq_c.T @ ... -> lhsT=wq chunks  [512 f, 2048 s]
  K^T                                    [512 f, 2048 s]
  V   = x @ wv_c                         [2048 s, 512 f]    (+ ones column)
  per head h, per key block kc:  S^T[k, q] = K^T_h[:,kc].T @ Q^T_h
  T = exp(S^T / sqrt(128))  (unnormalized softmax; scores are O(5) so no
      max-subtraction is needed in fp32), causal-masked
  per query block qb: O[q, d|sum] = sum_kc T_kc[:, qb].T @ [V_kc | 1]
  O /= sum  -> transpose via PE -> O^T [512 f, 2048 s]
  out partial = O^T.T @ wo_c             [2048 s, 2048 dmo]  fp32

Schedule (keeps the PE gapless and overlaps the ACT-bound exp):
  QK projections run r-major (4 concurrent PSUM groups over the feature
  dim) so compute follows the xt DMA stream with no startup stall; head-0
  score pass (pass1) plus head-1's narrow key-blocks are woven into the V
  projection loop (ACT idle there); in phase B a single rate-matched chunk
  queue interleaves the remaining pass1 exps with pass2 matmuls across all
  heads, with per-group transposes deferred one group so the PE never
  waits on the DVE normalize chain; w_o downloads during phase B; outputs
  are staged bf16 (host sums partials in fp32) halving store traffic.

Measured: 357us (session-start baseline) -> ~245us; bf16 PE busy is
~280us at the 2.4GHz cost model, ~202us at the ~3.3GHz silicon streams.
fp8 DoubleRow measured only 1.44x bf16 per MAC (157 TF/s vs 109) so
split-fp8 compensation is a net loss; exp cannot leave ACT; cross-
iteration overlap is SBUF-infeasible (needs ~235KB live vs 208KB).
"""

import math
import threading
from contextlib import ExitStack

import ml_dtypes
import numpy as np

import concourse.bass as bass
import concourse.mybir as mybir
import concourse.tile as tile
from concourse import bacc
from concourse.masks import make_identity

import os
_DT = os.environ.get("MHA_DTYPE", "bf16")
BF16 = mybir.dt.float16 if _DT == "fp16" else mybir.dt.bfloat16
F32 = mybir.dt.float32
NPBF16 = np.float16 if _DT == "fp16" else ml_dtypes.bfloat16

SEQ = 2048
DM = 2048
HEADS_PER_CORE = 4
F = 512  # features per core
P = 128
NKC = SEQ // P  # 16 key blocks
NR = DM // P  # 16 contraction chunks
SCALE = 1.0 / math.sqrt(128.0)

# compact T-buffer offsets: block kc covers q in [kc*128, 2048)
T_WIDTHS = [SEQ - kc * P for kc in range(NKC)]
T_OFFS = list(np.cumsum([0] + T_WIDTHS[:-1]))
T_TOTAL = int(np.sum(T_WIDTHS))  # 17408


def build_nc(iters: int = 1, rep_a: int = 1, rep_b: int = 1, rep_c: int = 1) -> bacc.Bacc:
    nc = bacc.Bacc("TRN2", num_devices=8)

    xt_h = nc.dram_tensor("xt", [DM, SEQ], BF16, kind="ExternalInput")
    wq_h = nc.dram_tensor("wq", [DM, F], BF16, kind="ExternalInput")
    wk_h = nc.dram_tensor("wk", [DM, F], BF16, kind="ExternalInput")
    wv_h = nc.dram_tensor("wv", [DM, F], BF16, kind="ExternalInput")
    wo_h = nc.dram_tensor("wo", [F, DM], BF16, kind="ExternalInput")
    tri_h = nc.dram_tensor("tri", [P, P], BF16, kind="ExternalInput")
    out_h = nc.dram_tensor("out", [SEQ, DM], BF16, kind="ExternalOutput")

    xt = xt_h.ap()
    wo_r = wo_h.ap().rearrange("(c p) n -> p c n", p=P)  # [128, 4, 2048]
    out_ap = out_h.ap()

    with tile.TileContext(nc) as tc, ExitStack() as octx:
        consts = octx.enter_context(tc.tile_pool(name="consts", bufs=1))
        ident = consts.tile([P, P], BF16)
        make_identity(nc, ident)
        tri_sb = consts.tile([P, P], BF16)

        for _ in range(iters):
            with ExitStack() as ictx:
                persist = ictx.enter_context(tc.tile_pool(name="persist", bufs=1))
                qt_sb = persist.tile([P, HEADS_PER_CORE, SEQ], BF16)
                kt_sb = persist.tile([P, HEADS_PER_CORE, SEQ], BF16)
                v_sb = persist.tile([P, NKC, HEADS_PER_CORE, P + 1], BF16)
                ot_sb = persist.tile([P, HEADS_PER_CORE, SEQ], BF16)
                t0_sb = persist.tile([P, T_TOTAL], BF16)
                # head-1's narrow key-blocks (kc >= NARKC) are exp'd during
                # the V window into this side buffer; the rest of head 1
                # lands in a phase-B tile
                NARKC = 8
                NAR_OFF = T_OFFS[NARKC]
                t1n_sb = persist.tile([P, T_TOTAL - NAR_OFF], BF16)

                # ones column for the fused softmax-denominator trick
                nc.vector.memset(v_sb[:, :, :, P : P + 1], 1.0)

                def pass1_chunks(h, resolve, psum_pool, ptag, kcs=None):
                    # chunk emitters for T = causal_mask(exp(S^T/sqrt(d)));
                    # resolve(kc) -> (tile, base_off); entries are
                    # (kc, act_cost, emit) with act_cost ~ exp width
                    chunks = []
                    for kc in kcs if kcs is not None else range(NKC):
                        w = T_WIDTHS[kc]
                        for c in range((w + 1023) // 1024):
                            def emit(kc=kc, c=c, w=w):
                                t_h, off = resolve(kc)
                                q0 = kc * P
                                lhsT = kt_sb[:, h, kc * P : (kc + 1) * P]
                                wc = min(1024, w - c * 1024)
                                ps = psum_pool.tile(
                                    [P, 1024], F32, tag=ptag, name="ps1"
                                )
                                for n in range((wc + 511) // 512):
                                    nw = min(512, wc - n * 512)
                                    o0 = c * 1024 + n * 512
                                    nc.tensor.matmul(
                                        ps[:, n * 512 : n * 512 + nw],
                                        lhsT,
                                        qt_sb[:, h, q0 + o0 : q0 + o0 + nw],
                                        start=True,
                                        stop=True,
                                    )
                                nc.scalar.activation(
                                    t_h[:, off + c * 1024 : off + c * 1024 + wc],
                                    ps[:, 0:wc],
                                    mybir.ActivationFunctionType.Exp,
                                    scale=SCALE,
                                )
                                if c == 0:
                                    nc.vector.tensor_mul(
                                        t_h[:, off : off + P],
                                        t_h[:, off : off + P],
                                        tri_sb,
                                    )
                            chunks.append((kc, min(1024, w - c * 1024), emit))
                    return chunks

                def mk_resolve(t_full, t_nar=None):
                    def resolve(kc):
                        if t_nar is not None and kc >= NARKC:
                            return t_nar, T_OFFS[kc] - NAR_OFF
                        return t_full, T_OFFS[kc]
                    return resolve

                res_t0 = mk_resolve(t0_sb)

                # ---------------- Phase A: projections ----------------
                for _ra in range(rep_a):
                  with ExitStack() as actx:
                    pa = actx.enter_context(tc.tile_pool(name="pa", bufs=1))
                    wpool = actx.enter_context(tc.tile_pool(name="wpool", bufs=2))
                    xt_sb = pa.tile([P, NR, SEQ], BF16)

                    # --- Q/K projections, r-major with 4 concurrent PSUM
                    # groups so the PE tracks the xt DMA stream ---
                    with ExitStack() as qkctx:
                        psq = qkctx.enter_context(
                            tc.tile_pool(name="psq", bufs=4, space="PSUM")
                        )
                        w_tiles = {}
                        w_rs = {}
                        for nm, w_h in (("wq", wq_h), ("wk", wk_h)):
                            w_rs[nm] = w_h.ap().rearrange("(r p) f -> p r f", p=P)
                            w_tiles[nm] = wpool.tile([P, NR, F], BF16, tag="w", name=nm)

                        # DMA emission order == SP queue order: first wq
                        # chunk, then xt half-0 rows interleaved with the
                        # rest of wq, then xt half-1, then wk.
                        wq_t, wk_t = w_tiles["wq"], w_tiles["wk"]
                        wq_r, wk_r = w_rs["wq"], w_rs["wk"]
                        nc.sync.dma_start(out=wq_t[:, 0:1, :], in_=wq_r[:, 0:1, :])
                        nc.sync.dma_start(
                            out=xt_sb[:, 0, 0:1024], in_=xt[0:P, 0:1024]
                        )
                        wq_plan = {1: (1, 4), 3: (4, 8), 5: (8, 12), 7: (12, 16)}
                        for r in range(1, NR):
                            if r in wq_plan:
                                a, b = wq_plan[r]
                                nc.sync.dma_start(
                                    out=wq_t[:, a:b, :], in_=wq_r[:, a:b, :]
                                )
                            nc.sync.dma_start(
                                out=xt_sb[:, r, 0:1024],
                                in_=xt[r * P : (r + 1) * P, 0:1024],
                            )
                        for r in range(NR):
                            nc.sync.dma_start(
                                out=xt_sb[:, r, 1024:2048],
                                in_=xt[r * P : (r + 1) * P, 1024:2048],
                            )
                        for rr in range(4):
                            nc.sync.dma_start(
                                out=wk_t[:, rr * 4 : (rr + 1) * 4, :],
                                in_=wk_r[:, rr * 4 : (rr + 1) * 4, :],
                            )

                        # psum[f_rel, s] = sum_r w[r,f].T @ xT[r, s]
                        for nm, dst in (("wq", qt_sb), ("wk", kt_sb)):
                            w_t = w_tiles[nm]
                            for half in range(2):
                                pqs = [
                                    psq.tile([P, 1024], F32, tag="psq", name="pq")
                                    for _ in range(HEADS_PER_CORE)
                                ]
                                for r in range(NR):
                                    for f in range(HEADS_PER_CORE):
                                        lhsT = w_t[:, r, f * P : (f + 1) * P]
                                        for sn in range(2):
                                            s0 = half * 1024 + sn * 512
                                            nc.tensor.matmul(
                                                pqs[f][:, sn * 512 : (sn + 1) * 512],
                                                lhsT,
                                                xt_sb[:, r, s0 : s0 + 512],
                                                start=(r == 0),
                                                stop=(r == NR - 1),
                                            )
                                for f in range(HEADS_PER_CORE):
                                    seg = dst[:, f, half * 1024 : (half + 1) * 1024]
                                    if f % 2 == 0:
                                        nc.vector.tensor_copy(seg, pqs[f])
                                    else:
                                        nc.scalar.copy(seg, pqs[f])

                    # --- V projection with head-0 pass1 woven in: exp runs
                    # on the otherwise-idle ACT engine while V matmuls keep
                    # the PE busy ---
                    with ExitStack() as vctx:
                        psv = vctx.enter_context(
                            tc.tile_pool(name="psv", bufs=2, space="PSUM")
                        )
                        psw = vctx.enter_context(
                            tc.tile_pool(name="psw", bufs=2, space="PSUM")
                        )
                        wv_r = wv_h.ap().rearrange("(r p) f -> p r f", p=P)
                        wv_t = wpool.tile([P, NR, F], BF16, tag="w")
                        for rr in range(4):
                            nc.sync.dma_start(
                                out=wv_t[:, rr * 4 : (rr + 1) * 4, :],
                                in_=wv_r[:, rr * 4 : (rr + 1) * 4, :],
                            )
                        nc.sync.dma_start(out=tri_sb, in_=tri_h.ap())

                        # weave head-0's full pass1 plus head-1's narrow
                        # key-blocks into the V window (ACT idle here)
                        h0_chunks = pass1_chunks(0, res_t0, psw, "psw") + pass1_chunks(
                            1, mk_resolve(None, t1n_sb), psw, "psw",
                            kcs=range(NARKC, NKC),
                        )
                        act_total = sum(c[1] for c in h0_chunks)
                        h0_emitted = 0
                        act_done = 0
                        # V: psum[s_rel, f] = sum_r xT[r, s].T @ wv[r, f]
                        for sm in range(NKC):
                            pv = psv.tile([P, F], F32, tag="psv")
                            for r in range(NR):
                                nc.tensor.matmul(
                                    pv,
                                    xt_sb[:, r, sm * P : (sm + 1) * P],
                                    wv_t[:, r, :],
                                    start=(r == 0),
                                    stop=(r == NR - 1),
                                )
                            nc.vector.tensor_copy(
                                v_sb[:, sm, :, 0:P],
                                pv.rearrange("p (h d) -> p h d", h=HEADS_PER_CORE),
                            )
                            # pace pass1 by ACT work so exp never backs up
                            want_act = (sm + 1) * act_total // NKC
                            while h0_emitted < len(h0_chunks) and act_done < want_act:
                                act_done += h0_chunks[h0_emitted][1]
                                h0_chunks[h0_emitted][2]()
                                h0_emitted += 1

                # ---------------- Phases B+C ----------------
                with ExitStack() as bcctx:
                    cw = bcctx.enter_context(tc.tile_pool(name="cw", bufs=1))
                    wo_sb = cw.tile([P, HEADS_PER_CORE, DM], BF16)
                    # SP queue is idle during phase B; download w_o now so
                    # phase C starts without a stall
                    nc.sync.dma_start(out=wo_sb, in_=wo_r)

                    # ---- Phase B: attention, heads software-pipelined ----
                    # slot h: pass2 of head h (reading its finished T) is
                    # interleaved with pass1 of head h+1 (matmuls + exp)
                    for _rb in range(rep_b):
                      with ExitStack() as bctx:
                        pb = bctx.enter_context(tc.tile_pool(name="pb", bufs=2))
                        pbo = bctx.enter_context(tc.tile_pool(name="pbo", bufs=3))
                        pss = bctx.enter_context(
                            tc.tile_pool(name="pss", bufs=2, space="PSUM")
                        )
                        pso = bctx.enter_context(
                            tc.tile_pool(name="pso", bufs=3, space="PSUM")
                        )
                        pst = bctx.enter_context(
                            tc.tile_pool(name="pst", bufs=1, space="PSUM")
                        )

                        # Global chunk queue across heads 1..3 (head 1's
                        # narrow blocks were exp'd during V): emission is
                        # rate-matched so ACT (exp) stays ~1.5us ahead of the
                        # PE, equalizing exp across the whole phase.
                        MM_NS = 0.4167  # PE ns/col
                        ACT_NS = 0.865  # ACT ns/col

                        t_tiles = {0: t0_sb}
                        resolves = {0: res_t0}
                        queue = []  # (head, kc, width, emit)
                        t1w = pb.tile([P, T_TOTAL], BF16, tag="T", name="t1w")
                        resolves[1] = mk_resolve(t1w, t1n_sb)
                        for kc, w, emit in pass1_chunks(
                            1, resolves[1], pss, "pss", kcs=range(NARKC)
                        ):
                            queue.append((1, kc, w, emit))
                        for hh in (2, 3):
                            tt = pb.tile([P, T_TOTAL], BF16, tag="T", name="tt")
                            resolves[hh] = mk_resolve(tt)
                            for kc, w, emit in pass1_chunks(
                                hh, resolves[hh], pss, "pss"
                            ):
                                queue.append((hh, kc, w, emit))

                        qi = 0
                        act_ns = 0.0
                        pe_ns = 0.0
                        act_tot = sum(c[2] for c in queue) * ACT_NS
                        pe_tot = (
                            HEADS_PER_CORE * (136 + 16) * P * MM_NS
                            + sum(c[2] for c in queue) * MM_NS
                        )
                        ratio = act_tot / pe_tot

                        def pump(h, qb, force=False):
                            nonlocal qi, act_ns, pe_ns
                            while qi < len(queue):
                                ch, ckc, w, emit = queue[qi]
                                due = ch < h or (ch == h and ckc <= qb)
                                paced = act_ns < pe_ns * ratio + 1500.0
                                # never run ahead by 2 heads: head h+2's T
                                # reuses the ring slot still read by pass2(h)
                                if ch > h + 1:
                                    break
                                if not (due or (paced and not force)):
                                    break
                                if force and not due:
                                    break
                                emit()
                                act_ns += w * ACT_NS
                                pe_ns += w * MM_NS
                                qi += 1

                        for h in range(HEADS_PER_CORE):
                            # pass 2: O accumulation + normalize + transpose;
                            # each group's transpose is deferred into the next
                            # group's matmul stream so the PE never waits on
                            # the DVE normalize chain
                            res_h = resolves[h]
                            pending = None  # (o_sb, qb) awaiting transpose
                            pt = None

                            def flush_transpose(h=h):
                                nonlocal pending, pt
                                if pending is None:
                                    return
                                o_prev, qb_prev = pending
                                i4 = qb_prev % 4
                                if i4 == 0:
                                    pt = pst.tile([P, 512], BF16, tag="pst", name="pt")
                                nc.tensor.transpose(
                                    pt[:, i4 * P : (i4 + 1) * P], o_prev, ident
                                )
                                if i4 == 3:
                                    g = qb_prev // 4
                                    nc.vector.tensor_copy(
                                        ot_sb[:, h, g * 512 : (g + 1) * 512], pt
                                    )
                                pending = None

                            for qb in range(NKC):
                                # anything this group reads must be emitted
                                pump(h, qb, force=True)
                                po = pso.tile([P, P + 1], F32, tag="pso")
                                for kc in range(qb + 1):
                                    t_h, base = res_h(kc)
                                    col = base + (qb - kc) * P
                                    nc.tensor.matmul(
                                        po,
                                        t_h[:, col : col + P],
                                        v_sb[:, kc, h, :],
                                        start=(kc == 0),
                                        stop=(kc == qb),
                                    )
                                    if kc == 0:
                                        flush_transpose()
                                pe_ns += (qb + 1) * P * MM_NS
                                recip = pbo.tile([P, 1], F32, tag="recip")
                                nc.vector.reciprocal(recip, po[:, P : P + 1])
                                o_sb = pbo.tile([P, P], BF16, tag="o")
                                nc.vector.tensor_scalar_mul(o_sb, po[:, 0:P], recip)
                                pending = (o_sb, qb)
                                pe_ns += P * MM_NS
                                pump(h, qb)
                            flush_transpose()

                    # ---------------- Phase C: output projection ----------------
                    for _rc in range(rep_c):
                      with ExitStack() as cctx:
                        stg = cctx.enter_context(tc.tile_pool(name="stg", bufs=3))
                        pco = cctx.enter_context(
                            tc.tile_pool(name="pco", bufs=3, space="PSUM")
                        )
                        for sm in range(NKC):
                            pos = [
                                pco.tile([P, 1024], F32, tag="pco", name="po")
                                for _ in range(2)
                            ]
                            for f in range(HEADS_PER_CORE):
                                lhsT = ot_sb[:, f, sm * P : (sm + 1) * P]
                                for nd in range(4):
                                    nc.tensor.matmul(
                                        pos[nd // 2][:, (nd % 2) * 512 : (nd % 2 + 1) * 512],
                                        lhsT,
                                        wo_sb[:, f, nd * 512 : (nd + 1) * 512],
                                        start=(f == 0),
                                        stop=(f == HEADS_PER_CORE - 1),
                                    )
                            stage = stg.tile([P, DM], BF16, tag="stage", name="stage")
                            # last row-block: 512-col pieces so the final
                            # copy+DMA tail is as short as possible
                            npc = 4 if sm == NKC - 1 else 2
                            wpc = DM // npc
                            for nd in range(npc):
                                seg = slice(nd * wpc, (nd + 1) * wpc)
                                src = pos[nd * wpc // 1024][:, nd * wpc % 1024 : nd * wpc % 1024 + wpc]
                                if (sm * npc + nd) % 2 == 0:
                                    nc.vector.tensor_copy(stage[:, seg], src)
                                else:
                                    nc.scalar.copy(stage[:, seg], src)
                                nc.sync.dma_start(
                                    out=out_ap[sm * P : (sm + 1) * P, seg],
                                    in_=stage[:, seg],
                                )

    nc.compile()
    return nc


def prep_in_maps(x, mask, w_q, w_k, w_v, w_o):
    """Host-side sharding: per-core input dicts (8 cores)."""
    x = np.asarray(x, dtype=np.float32)
    mask = np.asarray(mask, dtype=np.float32)
    w_q = np.asarray(w_q, dtype=np.float32)
    w_k = np.asarray(w_k, dtype=np.float32)
    w_v = np.asarray(w_v, dtype=np.float32)
    w_o = np.asarray(w_o, dtype=np.float32)

    # tri[k, q] = 1 where allowed (k <= q), from the mask's diagonal block
    tri = np.ascontiguousarray(
        (mask[:P, :P].T == 0.0).astype(NPBF16)
    )
    xts = [np.ascontiguousarray(x[b].T).astype(NPBF16) for b in range(2)]
    in_maps = []
    for c in range(8):
        b, j = divmod(c, 4)
        sl = slice(j * F, (j + 1) * F)
        in_maps.append(
            {
                "xt": xts[b],
                "wq": np.ascontiguousarray(w_q[:, sl]).astype(NPBF16),
                "wk": np.ascontiguousarray(w_k[:, sl]).astype(NPBF16),
                "wv": np.ascontiguousarray(w_v[:, sl]).astype(NPBF16),
                "wo": np.ascontiguousarray(w_o[sl, :]).astype(NPBF16),
                "tri": tri,
            }
        )
    return in_maps


def gather(results):
    """Sum the 4 partial outputs per batch element."""
    out = np.zeros((2, SEQ, DM), np.float32)
    for c in range(8):
        out[c // 4] += results[c]["out"]
    return out


_cache = threading.local()


def kernel(x, mask, w_q, w_k, w_v, w_o):
    from concourse.bass_utils import run_bass_kernel_spmd

    nc = getattr(_cache, "nc", None)
    if nc is None:
        nc = build_nc(1)
        _cache.nc = nc
    in_maps = prep_in_maps(x, mask, w_q, w_k, w_v, w_o)
    res = run_bass_kernel_spmd(nc, in_maps, core_ids=list(range(8)))
    return gather(res.results)
